# revision 1
# baseline (speedup 1.0000x reference)
"""BarlowTwins-style loss kernel for Trainium2 (raw Bass), 8-core SPMD.

Math: the reference materializes a (B, D, D) per-sample cross-correlation
tensor, but the loss algebraically reduces to O(B*D) work.  With
z1n/z2n the per-dim (batch-)normalized inputs and per-sample b:
    w    = z1n[b,:] * z2n[b,:]
    R    = sum(w);  P = sum(w^2);  Sa = sum(z1n^2);  Sv = sum(z2n^2)
    a    = z1n[b,b];  v = z2n[b,b];  d = a*v;  g2 = (d-1)^2
    u    = (a*z2n[b,:] - 1)^2;  Q = sum(u^2)
    on   = (P - 2R + D) - g2 + (g2-1)^2
    off  = (Sa - a^2)*Sv - P + d^2 + Q - g2^2
    loss = on + 0.005*off
         = -2R + (1-l)P + lQ + (1-l)g2^2 + l*d^2 - l*(a^2-Sa)*Sv
           - 3*g2 + (D+1)
  and with -3*g2 = -3(d-1)^2 = -3d^2 + 6d - 3 the final combine is a dot
  product of [R,P,Q] and [g2^2, d^2, f=(a^2-Sa)*Sv, 1, d] with constant
  coefficient rows (shipped in consts).

Sharding: data-parallel over batch.  Every core loads the full z1/z2 to
compute per-dim column sums/sumsq locally (an 8-core all-reduce has a
~10us latency floor; the redundant 1MB load is ~3us), then computes the
loss for its own 16 samples in a rearranged layout
[128 partitions = (sample, col-chunk), 128 free].

Key optimizations over the naive pipeline:
  - PE p-state warmup: ~10 dummy matmuls during the DMA phase keep the
    tensor engine ramping to 2.4GHz before the real matmuls arrive.
  - ACT table preload: dummy activation at t0 so the 1.3us table load
    happens during DMA, not on the critical path.
  - Stat matmuls write 4 partitions of ONE psum bank each (sums/sumsq),
    so PSUM->SBUF evacuation is 2 wide copies instead of 8 narrow ones,
    and a single SBUF->SBUF DMA scatters stats to [32,128].
  - bf16 element-wise phase (2x DVE throughput), consts shipped bf16.
  - DMA issue split across Sync+Scalar HWDGE queues.
  - Final per-sample combine as two coefficient dot products (accum_out).

Raw Bass (explicit semaphores): the walrus build in this container only
supports a single sync-wait per instruction; standalone wait_ge
instructions compose freely.
"""

import sys
from contextlib import ExitStack

import numpy as np

for _p in ("/opt/trn_rl_repo",):
    if _p not in sys.path:
        sys.path.append(_p)

import concourse.bass as bass
import concourse.mybir as mybir
from concourse.bass_utils import run_bass_kernel_spmd

B, D = 128, 1024
NCORES = 8
SPC = B // NCORES  # 16 samples per core
LAM = 0.005

FP = mybir.dt.float32
BF = mybir.dt.bfloat16
AF = mybir.ActivationFunctionType
AL = mybir.AluOpType

# cb (bf16) column layout
CB_Z1R = 0
CB_Z2R = 128
CB_AMASK = 256
CB_SEL = 384
CB_EYE = 512   # 8 x [128,8] one-hot-column tiles (stat matmul lhsT)
CB_SEL2 = 576
CB_TOTAL = 704
# cf (fp32) column layout
CF_GSEL = 0
CF_C1 = 16
CF_C2 = 19
CF_TOTAL = 32

N_WARM = 10


def build_program():
    nc = bass.Bass("TRN2", debug=False, num_devices=NCORES,
                   detect_race_conditions=False)

    z1_d = nc.dram_tensor("z1", [B, D], FP, kind="ExternalInput")
    z2_d = nc.dram_tensor("z2", [B, D], FP, kind="ExternalInput")
    cb_d = nc.dram_tensor("cb_hbm", [128, CB_TOTAL], BF, kind="ExternalInput")
    cf_d = nc.dram_tensor("cf_hbm", [128, CF_TOTAL], FP, kind="ExternalInput")
    loss_d = nc.dram_tensor("loss", [SPC, 1], FP, kind="ExternalOutput")

    ctx = ExitStack()
    with ctx:
        sem = {n: ctx.enter_context(nc.semaphore(n)) for n in
               ["dz0", "dz2", "dcb", "dcf", "scat", "scat2", "dout",
                "spe", "sv", "sa", "sg"]}

        def sb(name, shape, dtype=FP):
            return ctx.enter_context(nc.sbuf_tensor(name, shape, dtype))

        z = sb("z", [128, 2 * D])
        zb = sb("zb", [128, 2 * D], BF)
        sqb = sb("sqb", [128, 2 * D], BF)
        cb = sb("cb", [128, CB_TOTAL], BF)
        cf = sb("cf", [128, CF_TOTAL])
        statq8 = sb("statq8", [8, 512], BF)
        statS16 = sb("statS16", [16, 128], BF)
        statQ16 = sb("statQ16", [16, 128], BF)
        t1 = sb("t1", [16, 128], BF)
        var16 = sb("var16", [16, 128])
        stdv = sb("stdv", [16, 128])
        arec = sb("arec", [16, 128])
        acz = sb("acz", [16, 256], BF)
        mbf = sb("mbf", [16, 128], BF)
        a1c1 = sb("a1c1", [128, 256], BF)
        a2c2 = sb("a2c2", [128, 256], BF)
        tn1 = sb("tn1", [128, 128], BF)
        z1n = sb("z1n", [128, 128], BF)
        tn2 = sb("tn2", [128, 128], BF)
        z2n = sb("z2n", [128, 128], BF)
        w = sb("w", [128, 128], BF)
        u = sb("u", [128, 128], BF)
        junk_v = sb("junk_v", [128, 128], BF)
        junk_s = sb("junk_s", [128, 128], BF)
        colsD = sb("colsD", [128, 8])
        ones_b = sb("ones_b", [128, 1], BF)
        negone = sb("negone", [128, 1])
        a_sb = sb("a_sb", [128, 1])
        junk1c = sb("junk1c", [128, 1])
        fin2 = sb("fin2", [16, 8])
        djunk16 = sb("djunk16", [16, 8])
        qs = sb("qs", [16, 8])
        g2c = sb("g2c", [16, 1])
        ec = sb("ec", [16, 1])
        acc1c = sb("acc1c", [16, 1])
        acc2c = sb("acc2c", [16, 1])
        loss16 = sb("loss16", [16, 1])
        junkw = sb("junkw", [1, 4])
        junkw2 = sb("junkw2", [1, 4])

        # PSUM: 6 of 8 banks
        bankS = ctx.enter_context(nc.psum_tensor("bankS", [128, 512], FP))
        psBC1 = ctx.enter_context(nc.psum_tensor("psBC1", [128, 256], FP))
        psBC2 = ctx.enter_context(nc.psum_tensor("psBC2", [128, 256], FP))
        qfin = ctx.enter_context(nc.psum_tensor("qfin", [16, 8], FP))

        z1r = cb[:, CB_Z1R:CB_Z1R + 128]
        z2r = cb[:, CB_Z2R:CB_Z2R + 128]
        amask = cb[:, CB_AMASK:CB_AMASK + 128]
        selz1b = cb[0:16, CB_SEL:CB_SEL + 128]
        selz2b = cb[0:16, CB_SEL2:CB_SEL2 + 128]
        eye = [cb[:, CB_EYE + 8 * g:CB_EYE + 8 * g + 8] for g in range(8)]
        gsel = cf[:, CF_GSEL:CF_GSEL + 16]
        c1 = cf[0:16, CF_C1:CF_C1 + 3]
        c2 = cf[0:16, CF_C2:CF_C2 + 5]

        S_ = statS16[:, :]   # col sums (z1 chunks 0-7, z2 chunks 0-7)
        Qq = statQ16[:, :]   # col sumsq
        A_zb = acz[:, 0:128]
        C_zb = acz[:, 128:256]
        A1s = a1c1[:, 0:128]
        C1s = a1c1[:, 128:256]
        A2s = a2c2[:, 0:128]
        C2s = a2c2[:, 128:256]

        blk = [slice(i * 512, (i + 1) * 512) for i in range(4)]

        K1 = 1.0 / (B * (B - 1.0))
        K2 = 1.0 / (B - 1.0)

        with nc.Block() as block:

            @block.sync
            def _(sync):
                sync.dma_start(z[:, 0:1024], z1_d[:, :]).then_inc(sem["dz0"], 16)
                sync.dma_start(cf[:], cf_d[:]).then_inc(sem["dcf"], 16)
                # stats scatter (sums half): [4,512] -> [16,128] row-major
                sync.wait_ge(sem["sv"], 11)   # statq8 copy
                sync.dma_start(statS16[:], statq8[0:4, :]).then_inc(sem["scat"], 16)
                sync.wait_ge(sem["sv"], 48)   # loss16 done + drain
                sync.dma_start(loss_d[:], loss16[:]).then_inc(sem["dout"], 16)

            @block.scalar
            def _(act):
                act.dma_start(cb[:], cb_d[:]).then_inc(sem["dcb"], 16)
                act.dma_start(z[:, 1024:2048], z2_d[:, :]).then_inc(sem["dz2"], 16)
                # preload the ACT function table during the DMA phase
                act.square(junkw2[:], junkw[:]).then_inc(sem["sa"])                   # 1
                act.wait_ge(sem["sv"], 5)
                act.square(sqb[:, blk[0]], zb[:, blk[0]]).then_inc(sem["sa"])         # 2
                act.wait_ge(sem["sv"], 6)
                act.square(sqb[:, blk[1]], zb[:, blk[1]]).then_inc(sem["sa"])         # 3
                act.wait_ge(sem["sv"], 11)
                # stats scatter (sumsq half), issued from the ACT HWDGE queue
                act.dma_start(statQ16[:], statq8[4:8, :]).then_inc(sem["scat2"], 16)
                act.wait_ge(sem["sv"], 13)
                act.sqrt(stdv[:], var16[:]).then_inc(sem["sa"])                       # 4
                act.wait_ge(sem["sv"], 24)
                act.activation(u[:], z2n[:], AF.Square, bias=negone[:],
                               scale=a_sb[:]).then_inc(sem["sa"])                     # 5
                act.activation(junk_s[:], u[:], AF.Square,
                               accum_out=colsD[:, 2:3]).then_inc(sem["sa"])           # 6
                act.copy(junkw2[:], junkw[:]).then_inc(sem["sa"])                     # 7 (spacing)
                act.wait_ge(sem["sv"], 36)
                act.activation(g2c[:], fin2[:, 4:5], AF.Square,
                               bias=negone[0:16, :]).then_inc(sem["sa"])              # 8
                act.square(fin2[:, 0:1], g2c[:]).then_inc(sem["sa"])                  # 9

            @block.gpsimd
            def _(gp):
                gp.wait_ge(sem["dout"], 16)

            @block.vector
            def _(dve):
                dve.memset(negone[:], -1.0).then_inc(sem["sv"])                       # 1
                dve.memset(fin2[:, 3:4], 1.0).then_inc(sem["sv"])                     # 2
                dve.memset(junkw[:], 2.0).then_inc(sem["sv"])                         # 3
                dve.reciprocal(junkw2[:], junkw[:]).then_inc(sem["sv"])               # 4
                dve.wait_ge(sem["dz0"], 16)
                dve.tensor_copy(zb[:, blk[0]], z[:, blk[0]]).then_inc(sem["sv"])      # 5
                dve.tensor_copy(zb[:, blk[1]], z[:, blk[1]]).then_inc(sem["sv"])      # 6
                dve.wait_ge(sem["dz2"], 16)
                dve.tensor_copy(zb[:, blk[2]], z[:, blk[2]]).then_inc(sem["sv"])      # 7
                dve.tensor_copy(zb[:, blk[3]], z[:, blk[3]]).then_inc(sem["sv"])      # 8
                dve.scalar_tensor_tensor(
                    sqb[:, blk[2]], zb[:, blk[2]], 1.0, zb[:, blk[2]],
                    op0=AL.bypass, op1=AL.mult).then_inc(sem["sv"])                   # 9
                dve.scalar_tensor_tensor(
                    sqb[:, blk[3]], zb[:, blk[3]], 1.0, zb[:, blk[3]],
                    op0=AL.bypass, op1=AL.mult).then_inc(sem["sv"])                   # 10
                dve.wait_ge(sem["spe"], 8)
                dve.tensor_copy(statq8[:], bankS[0:8, :]).then_inc(sem["sv"])         # 11
                dve.wait_ge(sem["scat"], 16)
                dve.scalar_tensor_tensor(
                    t1[:], S_, K1, S_, op0=AL.mult, op1=AL.mult).then_inc(sem["sv"])  # 12
                dve.wait_ge(sem["scat2"], 16)
                dve.scalar_tensor_tensor(
                    var16[:], Qq, K2, t1[:],
                    op0=AL.mult, op1=AL.subtract).then_inc(sem["sv"])                 # 13
                dve.wait_ge(sem["sa"], 4)
                dve.reciprocal(arec[:], stdv[:]).then_inc(sem["sv"])                  # 14
                # two independent ops of spacing before arec is re-read
                dve.tensor_scalar_mul(mbf[:], S_, 1.0 / B).then_inc(sem["sv"])        # 15
                dve.tensor_scalar_mul(t1[:], S_, K2).then_inc(sem["sv"])              # 16 (spacing)
                dve.tensor_copy(A_zb, arec[:]).then_inc(sem["sv"])                    # 17
                dve.tensor_tensor(C_zb, mbf[:], A_zb, AL.mult).then_inc(sem["sv"])    # 18
                # normalize straight out of PSUM (one PSUM operand per op)
                dve.wait_ge(sem["spe"], 9)
                dve.tensor_tensor(tn1[:], z1r, psBC1[:, 0:128], AL.mult).then_inc(sem["sv"])  # 19
                dve.tensor_tensor(z1n[:], tn1[:], psBC1[:, 128:256],
                                  AL.subtract).then_inc(sem["sv"])                    # 20
                dve.scalar_tensor_tensor(
                    junk_v[:], z1n[:], 1.0, amask, op0=AL.bypass, op1=AL.mult,
                    accum_out=colsD[:, 3:4]).then_inc(sem["sv"])                      # 21
                dve.wait_ge(sem["spe"], 10)
                dve.tensor_tensor(tn2[:], z2r, psBC2[:, 0:128], AL.mult).then_inc(sem["sv"])  # 22
                dve.tensor_tensor(z2n[:], tn2[:], psBC2[:, 128:256],
                                  AL.subtract).then_inc(sem["sv"])                    # 23
                # 2 ops since the a-extract accum: safe to read colsD[:,3]
                dve.stream_shuffle(a_sb[:], colsD[:, 3:4],
                                   [8 * (i // 8) for i in range(32)]).then_inc(sem["sv"])  # 24
                dve.scalar_tensor_tensor(
                    w[:], z1n[:], 1.0, z2n[:], op0=AL.bypass, op1=AL.mult,
                    accum_out=colsD[:, 0:1]).then_inc(sem["sv"])                      # 25
                dve.scalar_tensor_tensor(
                    junk_v[:], w[:], 1.0, w[:], op0=AL.bypass, op1=AL.mult,
                    accum_out=colsD[:, 1:2]).then_inc(sem["sv"])                      # 26
                dve.scalar_tensor_tensor(
                    junk_v[:], z2n[:], 1.0, amask, op0=AL.bypass, op1=AL.mult,
                    accum_out=colsD[:, 4:5]).then_inc(sem["sv"])                      # 27
                dve.scalar_tensor_tensor(
                    junk_v[:], z1n[:], 1.0, z1n[:], op0=AL.bypass, op1=AL.mult,
                    accum_out=colsD[:, 5:6]).then_inc(sem["sv"])                      # 28
                dve.scalar_tensor_tensor(
                    junk_v[:], z2n[:], 1.0, z2n[:], op0=AL.bypass, op1=AL.mult,
                    accum_out=colsD[:, 6:7]).then_inc(sem["sv"])                      # 29
                dve.tensor_tensor(colsD[:, 7:8], colsD[:, 3:4], colsD[:, 3:4],
                                  AL.mult).then_inc(sem["sv"])                        # 30
                dve.tensor_tensor(junk1c[:], a_sb[:], a_sb[:],
                                  AL.mult).then_inc(sem["sv"])                        # 31 (spacing)
                dve.drain().then_inc(sem["sv"])                                       # 32
                # ---- finals ----
                # NOTE: same-engine RAW on tiny [16,1] tiles needs >=2 ops of
                # spacing (writes retire late); the chain is interleaved.
                dve.wait_ge(sem["spe"], 11)
                dve.tensor_copy(qs[:], qfin[:]).then_inc(sem["sv"])                   # 33
                dve.tensor_tensor(junk1c[:], colsD[:, 5:6], colsD[:, 6:7],
                                  AL.mult).then_inc(sem["sv"])                        # 34 (spacing)
                dve.tensor_tensor(junk1c[:], a_sb[:], a_sb[:],
                                  AL.mult).then_inc(sem["sv"])                        # 35 (spacing)
                dve.tensor_tensor(fin2[:, 4:5], qs[:, 3:4], qs[:, 4:5],
                                  AL.mult).then_inc(sem["sv"])                        # 36  d
                dve.tensor_tensor(ec[:], qs[:, 7:8], qs[:, 5:6],
                                  AL.subtract).then_inc(sem["sv"])                    # 37  a2-Sa
                dve.scalar_tensor_tensor(
                    djunk16[:, 0:3], qs[:, 0:3], 1.0, c1, op0=AL.bypass,
                    op1=AL.mult, accum_out=acc1c[:]).then_inc(sem["sv"])              # 38  acc1
                dve.tensor_tensor(fin2[:, 1:2], fin2[:, 4:5], fin2[:, 4:5],
                                  AL.mult).then_inc(sem["sv"])                        # 39  d^2
                dve.tensor_tensor(fin2[:, 2:3], ec[:], qs[:, 6:7],
                                  AL.mult).then_inc(sem["sv"])                        # 40  f
                dve.tensor_tensor(junk1c[:], a_sb[:], a_sb[:],
                                  AL.mult).then_inc(sem["sv"])                        # 41 (spacing)
                dve.tensor_tensor(junk1c[:], colsD[:, 5:6], colsD[:, 6:7],
                                  AL.mult).then_inc(sem["sv"])                        # 42 (spacing)
                dve.wait_ge(sem["sa"], 9)
                dve.scalar_tensor_tensor(
                    djunk16[:, 0:5], fin2[:, 0:5], 1.0, c2, op0=AL.bypass,
                    op1=AL.mult, accum_out=acc2c[:]).then_inc(sem["sv"])              # 43  acc2
                dve.tensor_tensor(junk1c[:], a_sb[:], a_sb[:],
                                  AL.mult).then_inc(sem["sv"])                        # 44 (spacing)
                dve.tensor_tensor(junk1c[:], colsD[:, 5:6], colsD[:, 6:7],
                                  AL.mult).then_inc(sem["sv"])                        # 45 (spacing)
                dve.drain().then_inc(sem["sv"])                                       # 46
                dve.tensor_tensor(loss16[:], acc1c[:], acc2c[:],
                                  AL.add).then_inc(sem["sv"])                         # 47
                dve.drain().then_inc(sem["sv"])                                       # 48

            @block.tensor
            def _(pe):
                # col sums accumulate into bankS rows 0-3 and sumsq into
                # rows 4-7 (one-hot lhsT columns place each block's result
                # in its own partition of one accumulation group).
                pe.wait_ge(sem["dcb"], 16)
                pe.wait_ge(sem["sv"], 5)
                pe.matmul(bankS[0:8, :], eye[0], zb[:, blk[0]], start=True,
                          stop=False, skip_group_check=True).then_inc(sem["spe"])     # 1
                pe.wait_ge(sem["sv"], 6)
                pe.matmul(bankS[0:8, :], eye[1], zb[:, blk[1]], start=False,
                          stop=False, skip_group_check=True).then_inc(sem["spe"])     # 2
                pe.wait_ge(sem["sa"], 2)
                pe.matmul(bankS[0:8, :], eye[4], sqb[:, blk[0]], start=False,
                          stop=False, skip_group_check=True).then_inc(sem["spe"])     # 3
                pe.wait_ge(sem["sa"], 3)
                pe.matmul(bankS[0:8, :], eye[5], sqb[:, blk[1]], start=False,
                          stop=False, skip_group_check=True).then_inc(sem["spe"])     # 4
                pe.wait_ge(sem["sv"], 7)
                pe.matmul(bankS[0:8, :], eye[2], zb[:, blk[2]], start=False,
                          stop=False, skip_group_check=True).then_inc(sem["spe"])     # 5
                pe.wait_ge(sem["sv"], 8)
                pe.matmul(bankS[0:8, :], eye[3], zb[:, blk[3]], start=False,
                          stop=False, skip_group_check=True).then_inc(sem["spe"])     # 6
                pe.wait_ge(sem["sv"], 9)
                pe.matmul(bankS[0:8, :], eye[6], sqb[:, blk[2]], start=False,
                          stop=False, skip_group_check=True).then_inc(sem["spe"])     # 7
                pe.wait_ge(sem["sv"], 10)
                pe.matmul(bankS[0:8, :], eye[7], sqb[:, blk[3]], start=False,
                          stop=True, skip_group_check=True).then_inc(sem["spe"])      # 8
                # broadcast A||C to the (sample, chunk) layout
                pe.wait_ge(sem["sv"], 18)
                pe.matmul(psBC1[:], selz1b, acz[:], start=True,
                          stop=True).then_inc(sem["spe"])                             # 9
                pe.matmul(psBC2[:], selz2b, acz[:], start=True, stop=True,
                          skip_group_check=True).then_inc(sem["spe"])                 # 10
                # group-reduce: collapse 8 chunk-rows per sample
                pe.wait_ge(sem["dcf"], 16)
                pe.wait_ge(sem["sv"], 32)
                pe.wait_ge(sem["sa"], 7)
                pe.matmul(qfin[:], gsel, colsD[:], start=True,
                          stop=True).then_inc(sem["spe"])                             # 11

    return nc


def _host_inputs(z1, z2):
    """Per-core input maps (sharding glue)."""
    import ml_dtypes

    z1 = np.ascontiguousarray(z1, np.float32)
    z2 = np.ascontiguousarray(z2, np.float32)

    cb_base = np.zeros((128, CB_TOTAL), np.float32)
    for m in range(128):
        cb_base[m % 8, CB_SEL + m] = 1.0        # selz1b (reads A/C rows 0-7)
        cb_base[8 + m % 8, CB_SEL2 + m] = 1.0   # selz2b (reads A/C rows 8-15)
    for g in range(8):
        cb_base[:, CB_EYE + 9 * g] = 1.0        # eye[g]: ones in column g

    cf_base = np.zeros((128, CF_TOTAL), np.float32)
    for m in range(128):
        cf_base[m, CF_GSEL + m // 8] = 1.0      # gsel
    cf_base[0:16, CF_C1:CF_C1 + 3] = np.array(
        [-2.0, 1.0 - LAM, LAM], np.float32)
    cf_base[0:16, CF_C2:CF_C2 + 5] = np.array(
        [1.0 - LAM, LAM - 3.0, -LAM, float(D - 2), 6.0], np.float32)

    in_maps = []
    for c in range(NCORES):
        rows = slice(c * SPC, (c + 1) * SPC)
        cbc = cb_base.copy()
        cbc[:, CB_Z1R:CB_Z1R + 128] = z1[rows].reshape(128, 128)
        cbc[:, CB_Z2R:CB_Z2R + 128] = z2[rows].reshape(128, 128)
        for s in range(SPC):
            cbc[s * 8, CB_AMASK + c * SPC + s] = 1.0
        in_maps.append({
            "z1": z1, "z2": z2,
            "cb_hbm": np.ascontiguousarray(cbc.astype(ml_dtypes.bfloat16)),
            "cf_hbm": np.ascontiguousarray(cf_base),
        })
    return in_maps


_cached_nc = None


def run(z1, z2, trace=False, **kwargs):
    global _cached_nc
    if _cached_nc is None:
        _cached_nc = build_program()
    in_maps = _host_inputs(z1, z2)
    res = run_bass_kernel_spmd(
        _cached_nc, in_maps, core_ids=list(range(NCORES)), trace=trace, **kwargs)
    out = np.concatenate([res.results[c]["loss"][:, 0] for c in range(NCORES)])
    return out.astype(np.float32), res


def kernel(z1, z2):
    out, _ = run(z1, z2, trace=False)
    return out

